# revision 1
# baseline (speedup 1.0000x reference)
"""Contrastive loss kernel for Trainium2 (8 NeuronCores).

loss = mean((sim.sum(-1) - diag) / T) with sim = n @ n.T, n = x/||x||
     = (||sum_i n_i||^2 - sum_i ||n_i||^2) / (N*T)
     = (||s||^2 - N) / (N*T)          with s = sum_i x_i / ||x_i||

Each core takes a [2048, 512] row shard (16 row-tiles of [128, 512]).
Row sum-of-squares alternates between VectorE (bn_stats -> D*(var+mean^2),
even tiles) and ScalarE (Square activation with accum_out, odd tiles) to
balance the engines; rnorm = reciprocal(sqrt(ss)) with the sqrt batched
per group. The partial s_local = sum_i rnorm_i * x_i is 16 PE matmuls
(lhsT = rnorm [128,1], rhs = x tile [128,512], float32r for full-rate PE)
accumulated in one PSUM bank, copied to SBUF, and DMA'd out per core as
a [1, 512] vector. The host sums the 8 partials and applies the scalar
epilogue (all-reduce of a [D] vector + scalar math).
"""

import numpy as np

import concourse.bass as bass
import concourse.bacc as bacc
import concourse.tile as tile
from concourse import mybir
from concourse.bass_utils import run_bass_kernel_spmd

N = 16384
D = 512
NCORES = 8
ROWS = N // NCORES   # 2048 rows per core
P = 128              # SBUF partitions
NTILES = ROWS // P   # 16 row-tiles per core
GROUPS = (4, 4, 2, 1, 1, 1, 1, 1, 1)   # rsqrt batch sizes (sum = NTILES)
TEMPERATURE = 0.5

F32 = mybir.dt.float32
F32R = mybir.dt.float32r
SQUARE = mybir.ActivationFunctionType.Square

_NC = None


def _build_nc() -> bass.Bass:
    nc = bacc.Bacc(None)
    x_in = nc.declare_dram_parameter("x", [ROWS, D], F32R, isOutput=False)
    s_out = nc.declare_dram_parameter("s", [1, D], F32, isOutput=True)
    x_t = x_in.rearrange("(t p) d -> p t d", p=P)

    with tile.TileContext(nc) as tc:
        with (
            tc.tile_pool(name="xs", bufs=NTILES) as xs_pool,
            tc.tile_pool(name="sq", bufs=2) as sq_pool,
            tc.tile_pool(name="bn", bufs=4) as bn_pool,
            tc.tile_pool(name="st", bufs=16) as st_pool,
            tc.tile_pool(name="acc", bufs=1, space="PSUM") as psum_pool,
            tc.tile_pool(name="one", bufs=1) as one_pool,
        ):
            acc = psum_pool.tile([1, D], F32)

            xt = [None] * NTILES   # float32r views (PE operands)
            xf = [None] * NTILES   # float32 views of the same bytes (stats)
            for i in range(NTILES):
                x2 = xs_pool.tile([P, D], F32R)
                nc.sync.dma_start(out=x2, in_=x_t[:, i, :])
                xt[i] = x2[:, :]
                xf[i] = x2[:, :].bitcast(F32)

            def emit_stats(t, ss_col):
                if t % 2 == 1:
                    # ScalarE: ss = sum_d x^2 via Square + accumulate
                    sq = sq_pool.tile([P, D], F32)
                    nc.scalar.activation(
                        out=sq, in_=xf[t], func=SQUARE, accum_out=ss_col
                    )
                else:
                    # VectorE: ss = D*(var + mean^2) == sum_d x^2
                    bn6 = bn_pool.tile([P, 6], F32, tag="bn6")
                    nc.vector.bn_stats(out=bn6, in_=xf[t])
                    mv = bn_pool.tile([P, 2], F32, tag="mv")
                    nc.vector.bn_aggr(out=mv, in_=bn6)
                    m2 = bn_pool.tile([P, 1], F32, tag="m2")
                    nc.vector.tensor_mul(m2, mv[:, 0:1], mv[:, 0:1])
                    nc.vector.tensor_scalar(
                        out=ss_col,
                        in0=m2,
                        scalar1=mv[:, 1:2],
                        scalar2=float(D),
                        op0=mybir.AluOpType.add,
                        op1=mybir.AluOpType.mult,
                    )

            rn = [None] * NTILES
            base = 0
            for gsz in GROUPS:
                tiles = range(base, base + gsz)
                base += gsz
                ss = st_pool.tile([P, gsz], F32, tag="ss")
                for j, t in enumerate(tiles):
                    emit_stats(t, ss[:, j : j + 1])
                nc.scalar.sqrt(out=ss, in_=ss)
                r = st_pool.tile([P, gsz], F32R, tag="rn")
                with nc.allow_low_precision(reason="fp32r rounding for PE operands"):
                    nc.vector.reciprocal(out=r, in_=ss)
                for j, t in enumerate(tiles):
                    rn[t] = r[:, j : j + 1]

            for i in range(NTILES):
                nc.tensor.matmul(
                    acc,
                    lhsT=rn[i],
                    rhs=xt[i],
                    start=(i == 0),
                    stop=(i == NTILES - 1),
                )

            res = one_pool.tile([1, D], F32)
            nc.scalar.copy(out=res, in_=acc)
            nc.sync.dma_start(out=s_out[:, :], in_=res)

    nc.finalize()
    return nc


def _run(x: np.ndarray, trace: bool = False):
    global _NC
    if _NC is None:
        _NC = _build_nc()
    x = np.ascontiguousarray(np.asarray(x, dtype=np.float32)).reshape(NCORES, ROWS, D)
    in_maps = [{"x": x[c]} for c in range(NCORES)]
    out = run_bass_kernel_spmd(_NC, in_maps, core_ids=list(range(NCORES)), trace=trace)
    s = np.zeros(D, dtype=np.float64)
    for r in out.results:
        s += r["s"].reshape(D).astype(np.float64)
    loss = (float(s @ s) - float(N)) / (N * TEMPERATURE)
    return np.asarray(loss, dtype=np.float32), out


def kernel(x: np.ndarray) -> np.ndarray:
    loss, _ = _run(x)
    return loss



# revision 21
# speedup vs baseline: 1.3301x; 1.3301x over previous
"""Contrastive loss kernel for Trainium2 (8 NeuronCores).

loss = mean((sim.sum(-1) - diag) / T) with sim = n @ n.T, n = x/||x||
     = (||s||^2 - N) / (N*T)          with s = sum_i x_i / ||x_i||

Each core takes a [2048, 512] row shard, shipped as fp16 packed
[128, 16, 512] (partition p holds rows 16p..16p+15), streamed in eight
2-segment DMAs so stats pipeline behind the transfers.  Per 512-dim
segment t the row sum-of-squares ss[:, t] is computed on DVE
(2x-mode tensor_mul square + 4x-mode tensor_scalar add-reduce) or on
ACT (Square activation with accum_out), interleaved to balance the two
engines.  rn = sqrt(1/ss) via DVE reciprocal + ACT sqrt; a dummy sqrt
on a const AP runs first so a single activation-table load (the
sqrt_and_others set, which also contains square) covers the kernel.
The weighted row sum s = sum_t x_t^T @ rn_t runs on the PE as 64 tiny
matmuls with the x tile as the 128x128 stationary operand and rn[:, t]
as the 1-column moving operand; chunk c of s accumulates in its own
PSUM bank (each accumulation group needs its own 2KB zero region).
The [128, 4] result is copied to SBUF (split across DVE/ACT) and DMA'd
out per core; the host sums the 8 partials and applies the scalar
epilogue.
"""

import numpy as np

import concourse.bass as bass
import concourse.bacc as bacc
import concourse.tile as tile
from concourse import mybir
from concourse.bass_utils import run_bass_kernel_spmd

N = 16384
D = 512
NCORES = 8
ROWS = N // NCORES    # 2048 rows per core
P = 128               # SBUF partitions
NSEG = ROWS // P      # 16 segments of [128, 512] per core
DCH = D // P          # 4 psum chunks of 128 dims
TEMPERATURE = 0.5

# DMA chunking: segments per input DMA (sum = NSEG)
DMA_CHUNKS = (2, 2, 2, 2, 2, 2, 2, 2)
# engine per segment: 'w' = DVE square+accum, 'a' = ACT square+accum
STATS_ENG = "wawawawawawawwaw"
# rsqrt batches (sum = NSEG)
RSQ_GROUPS = (12, 3, 1)

F32 = mybir.dt.float32
F16 = mybir.dt.float16
SQUARE = mybir.ActivationFunctionType.Square

_NC = None


def _build_nc(dma_chunks=None, stats_eng=None, rsq_groups=None) -> bass.Bass:
    dma_chunks = dma_chunks or DMA_CHUNKS
    stats_eng = stats_eng or STATS_ENG
    rsq_groups = rsq_groups or RSQ_GROUPS
    nc = bacc.Bacc(None)
    x_in = nc.declare_dram_parameter("x", [P, NSEG, D], F16, isOutput=False)
    s_out = nc.declare_dram_parameter("s", [P, DCH], F32, isOutput=True)

    with tile.TileContext(nc) as tc:
        with (
            tc.tile_pool(name="xs", bufs=1) as xs_pool,
            tc.tile_pool(name="scr", bufs=1) as scr_pool,
            tc.tile_pool(name="st", bufs=1) as st_pool,
            tc.tile_pool(name="acc", bufs=1, space="PSUM") as psum_pool,
        ):
            xt = xs_pool.tile([P, NSEG, D], F16, tag="x")
            scr_v = scr_pool.tile([P, D], F16, tag="scr_v")
            scr_a = scr_pool.tile([P, D], F16, tag="scr_a")
            scr_d = scr_pool.tile([P, D], F16, tag="scr_d")
            ss = st_pool.tile([P, NSEG], F32, tag="ss")
            ri = st_pool.tile([P, NSEG], F32, tag="ri")
            rn = st_pool.tile([P, NSEG], F16, tag="rn")
            # one full PSUM bank (2KB zero region) per accumulation group
            acc0 = psum_pool.tile([P, 512], F32, tag="acc0")
            acc1 = psum_pool.tile([P, 512], F32, tag="acc1")
            acc2 = psum_pool.tile([P, 512], F32, tag="acc2")
            acc3 = psum_pool.tile([P, 512], F32, tag="acc3")
            accs = [acc0, acc1, acc2, acc3]
            res = st_pool.tile([P, DCH], F32, tag="res")
            dum = st_pool.tile([P, 1], F32, tag="dum")

            # Dummy sqrt first so the single activation table loaded covers
            # both Sqrt and Square (sqrt_and_others); runs under the DMA head.
            # Input is a const AP so the sqrt has no cross-engine deps.
            nc.scalar.sqrt(out=dum, in_=nc.const_aps.tensor(0.0, (P, 1)))

            base = 0
            for csz in dma_chunks:
                nc.sync.dma_start(
                    out=xt[:, base : base + csz, :],
                    in_=x_in[:, base : base + csz, :],
                )
                base += csz

            def emit_stats(t):
                if stats_eng[t] == "a":
                    nc.scalar.activation(
                        out=scr_a,
                        in_=xt[:, t, :],
                        func=SQUARE,
                        accum_out=ss[:, t : t + 1],
                    )
                else:
                    # DVE two-op: 2x-mode square then 4x-mode add-reduce
                    nc.vector.tensor_mul(scr_v, xt[:, t, :], xt[:, t, :])
                    nc.vector.tensor_scalar(
                        out=scr_d,
                        in0=scr_v,
                        scalar1=1.0,
                        scalar2=0.0,
                        op0=mybir.AluOpType.mult,
                        op1=mybir.AluOpType.add,
                        accum_out=ss[:, t : t + 1],
                    )

            base = 0
            for gsz in rsq_groups:
                lo, hi = base, base + gsz
                base += gsz
                for t in range(lo, hi):
                    emit_stats(t)
                nc.vector.reciprocal(out=ri[:, lo:hi], in_=ss[:, lo:hi])
                with nc.allow_low_precision(reason="fp16 rnorm for PE rhs"):
                    nc.scalar.sqrt(out=rn[:, lo:hi], in_=ri[:, lo:hi])
                for t in range(lo, hi):
                    for c in range(DCH):
                        nc.tensor.matmul(
                            accs[c][:, 0:1],
                            lhsT=xt[:, t, c * P : (c + 1) * P],
                            rhs=rn[:, t : t + 1],
                            start=(t == 0),
                            stop=(t == NSEG - 1),
                        )

            # gather the 4 bank columns into res; split DVE/ACT for overlap
            nc.vector.tensor_scalar_mul(res[:, 0:1], accs[0][:, 0:1], 1.0)
            nc.scalar.copy(out=res[:, 2:3], in_=accs[2][:, 0:1])
            nc.vector.tensor_scalar_mul(res[:, 1:2], accs[1][:, 0:1], 1.0)
            nc.scalar.copy(out=res[:, 3:4], in_=accs[3][:, 0:1])
            nc.sync.dma_start(out=s_out[:, :], in_=res)

    nc.finalize()
    return nc


def _shard(x: np.ndarray) -> list[dict]:
    xh = np.ascontiguousarray(x, dtype=np.float32).astype(np.float16)
    xh = xh.reshape(NCORES, P, NSEG, D)
    return [{"x": xh[c]} for c in range(NCORES)]


def _run(x: np.ndarray, trace: bool = False):
    global _NC
    if _NC is None:
        _NC = _build_nc()
    out = run_bass_kernel_spmd(
        _NC, _shard(x), core_ids=list(range(NCORES)), trace=trace
    )
    s = np.zeros(D, dtype=np.float64)
    for r in out.results:
        # res[p, c] = s[c*128 + p]
        s += r["s"].reshape(P, DCH).T.reshape(D).astype(np.float64)
    loss = (float(s @ s) - float(N)) / (N * TEMPERATURE)
    return np.asarray(loss, dtype=np.float32), out


def kernel(x: np.ndarray) -> np.ndarray:
    loss, _ = _run(x)
    return loss
